# revision 22
# baseline (speedup 1.0000x reference)
"""Trainium2 Bass kernel for a dense transformer block (pre-LN, causal MHA + FFN).

Reference computation (per batch element b, T=64 tokens, D=384 features):
    h   = LN(x)*g1 + be1
    q,k,v per-head linears; scores = q k^T / sqrt(48); causal softmax
    attn = probs @ v, concat heads, @ wo + bo
    h    = h + attn              (residual from the *normed* x)
    h2   = LN(h)*g2 + be2
    out  = h2 + relu(h2@w1+b1)@w2 + b2

Sharding: pure data parallel over batch (2048 -> 256 per core, 8 cores),
params replicated; the same single-core program runs SPMD on all 8 cores.

v3 design (vs the fp32/f32r baseline):
  - ALL matmul operands bf16 (PSUM accumulation fp32): 1 cyc/row at any
    output width (fp32 = 4 cyc/row; f32r = 4 cyc/row below 256-wide).
  - NB=8 batch elems per tile (512 tokens): wider moving dims amortize the
    per-instruction issue cost (bf16 matmuls lower to Ldweights+Matmult
    pairs on the PE sequencer) and halve DMA/LN/copy instruction counts.
  - LN rstd via Quake rsqrt (bitcast + shift + 1 Newton step) entirely on
    DVE: the Act engine then only runs Exp/Copy/Relu which share one
    activation table (Sqrt shares no table with Exp -> the baseline
    reloaded act tables twice per tile, 1.28us each).
  - transposes on PE (bf16 transpose = 1 cyc/row), pairs merged into one
    PSUM tile so each [128,512] needs a single PSUM->SBUF copy.
  - V "half-swapped" copy on DVE (bf16 SBUF->SBUF runs at 2-4x).
  - PSUM->SBUF copies balanced across Act / DVE / GpSimd.

Per-core layout (tile = NB=8 batch elems = 512 tokens):
  - token-major [128 tok, feat] for LayerNorm (bn_stats) + residual adds
  - feature-major [feat 128-chunk, tok] for matmul inputs
  - heads 64-padded so per-(batch,head) matmuls sit at partition base 0/64
    (legal 64x64 PE array tiling; tile_position inferred from AP bases).
"""

import sys

sys.path.insert(0, "/opt/trn_rl_repo")

import numpy as np

import concourse.bass as bass
import concourse.tile as tile
from concourse import mybir

# ---- problem constants (hardcoded per contract) ----
B_TOTAL = 2048
T = 64
D = 384
H = 8
E = 48  # head size
EP = 64  # padded head size
F = 4 * D  # ffn hidden 1536
N_CORES = 8
B_CORE = B_TOTAL // N_CORES  # 256
LN_EPS = 1e-5
INV_SQRT_E = float(E) ** -0.5

NB = 8  # batch elems per tile
NT = NB * T  # tokens per tile = 512
KC = D // 128  # 3 contraction chunks for D
FC = F // 128  # 12 chunks for ffn hidden
TC = NT // 128  # 4 token chunks per tile
DP = H * EP  # padded qkv width 512

F32 = mybir.dt.float32
F32R = mybir.dt.float32r
I32 = mybir.dt.int32
BF16 = mybir.dt.bfloat16

QUAKE_MAGIC = 0x5F3759DF


def build_body(tc, aps, b_core):
    from contextlib import ExitStack

    ctx = ExitStack()
    nc = tc.nc
    n_tiles = b_core * T // NT

    x_dr = aps["x"].rearrange("b t d -> (b t) d")
    out_dr = aps["out"].rearrange("b t d -> (b t) d")

    AF = mybir.ActivationFunctionType
    OP = mybir.AluOpType
    flags = aps["flags"]

    singles = ctx.enter_context(tc.tile_pool(name="singles", bufs=1))

    def load_const(name, shape, src_ap, dt=BF16):
        t_ = singles.tile(list(shape), dt, name=f"sb_{name}")
        nc.sync.dma_start(out=t_, in_=src_ap)
        return t_

    ident = load_const("ident", [128, 128], aps["ident"])
    mask = load_const("mask", [128, 8 * T], aps["mask"])
    # dense-path weights in f32r: self-loading matmuls (no Ldweights on the
    # PE sequencer) and 1 cyc/row since every dense output is >= 384 wide.
    wqk = {
        (qi, k, ch): load_const(
            f"wqk{qi}{k}{ch}", [128, 128], aps["wqk"][qi, k, ch], F32R
        )
        for qi in range(2)
        for k in range(KC)
        for ch in range(4)
    }
    wv = {k: load_const(f"wv{k}", [128, DP], aps["wv"][k], F32R) for k in range(KC)}
    wo = {ch: load_const(f"wo{ch}", [128, D], aps["wo"][ch], F32R) for ch in range(4)}
    w1 = {
        (k, f): load_const(f"w1{k}_{f}", [128, 128], aps["w1"][k, f], F32R)
        for k in range(KC)
        for f in range(FC)
    }
    w2 = {f: load_const(f"w2{f}", [128, D], aps["w2"][f], F32R) for f in range(FC)}
    # bias/affine constants only when the problem actually uses them
    bqk = load_const("bqk", [128, 8], aps["bqk"], F32) if flags["bqk"] else None
    bv_b = load_const("bv_b", [128, DP], aps["bv_b"], F32) if flags["bv"] else None
    b1c = load_const("b1c", [128, FC], aps["b1c"], F32) if flags["b1"] else None
    g1_b = load_const("g1_b", [128, D], aps["g1_b"], F32) if flags["g1be1"] else None
    be1_b = load_const("be1_b", [128, D], aps["be1_b"], F32) if flags["g1be1"] else None
    g2_b = load_const("g2_b", [128, D], aps["g2_b"], F32) if flags["g2be2"] else None
    be2_b = load_const("be2_b", [128, D], aps["be2_b"], F32) if flags["g2be2"] else None
    bo_b = load_const("bo_b", [128, D], aps["bo_b"], F32) if flags["bo"] else None
    b2_b = load_const("b2_b", [128, D], aps["b2_b"], F32) if flags["b2"] else None
    quake = load_const("quake", [128, 8], aps["quake"], I32)  # cols 0:4=magic, 4:8=1

    pool = lambda nm, n, **kw: ctx.enter_context(tc.tile_pool(name=nm, bufs=n, **kw))
    ps = pool("ps", 4, space="PSUM")  # 1 tag -> 4 banks
    ps_at = pool("ps_at", 1, space="PSUM")  # 4 tags -> 4 banks
    p_x = pool("p_x", 4)
    p_h = pool("p_h", 6)
    p_hT = pool("p_hT", 1)  # 3 tags; writer phase directly precedes readers
    p_qk = pool("p_qk", 2)  # 8 tags
    p_v = pool("p_v", 2)  # 8 tags
    p_sm = pool("p_sm", 5)  # 4 ex tiles alive across pass1/pass2 + pipeline
    p_pt = pool("p_pt", 3)
    p_at = pool("p_at", 1)  # 4 tags
    p_hr = pool("p_hr", 6)
    p_h2 = pool("p_h2", 6)
    p_rel = pool("p_rel", 1)  # 12 tags
    p_out = pool("p_out", 3)
    p_st = pool("p_st", 8)

    def rsqrt_dve(ve_view, n, tag):
        """rstd[128, n] = 1/sqrt(ve) via Quake bitcast + 1 Newton step on DVE.

        Keeps Sqrt off the Act engine so Exp/Copy/Relu share one act table.
        ve in [~0.5, ~2] here, so one Newton step reaches ~2e-3 rel err.
        """
        ve = p_st.tile([128, n], F32, tag=f"q0{tag}", name=f"q0{tag}")
        nc.vector.tensor_scalar_add(out=ve, in0=ve_view, scalar1=LN_EPS)
        t1 = p_st.tile([128, n], I32, tag=f"q1{tag}", name=f"q1{tag}")
        nc.vector.tensor_tensor(
            out=t1, in0=ve.bitcast(I32), in1=quake[:, 4 : 4 + n], op=OP.logical_shift_right
        )
        y0 = p_st.tile([128, n], F32, tag=f"q2{tag}", name=f"q2{tag}")
        nc.vector.tensor_tensor(
            out=y0.bitcast(I32), in0=quake[:, 0:n], in1=t1, op=OP.subtract
        )
        a = p_st.tile([128, n], F32, tag=f"q3{tag}", name=f"q3{tag}")
        nc.vector.tensor_mul(out=a, in0=y0, in1=y0)
        b = p_st.tile([128, n], F32, tag=f"q4{tag}", name=f"q4{tag}")
        nc.vector.scalar_tensor_tensor(
            out=b, in0=a, scalar=-0.5, in1=ve, op0=OP.mult, op1=OP.mult
        )
        c = p_st.tile([128, n], F32, tag=f"q5{tag}", name=f"q5{tag}")
        nc.vector.tensor_scalar_add(out=c, in0=b, scalar1=1.5)
        rstd = p_st.tile([128, n], F32, tag=f"q6{tag}", name=f"q6{tag}")
        nc.vector.tensor_mul(out=rstd, in0=y0, in1=c)
        return rstd

    def layernorm_group(x_ts, g_b, be_b, gb_nontriv, pool, tag):
        """LN over TC token chunks; per-chunk stats, one batched rstd."""
        mv = p_st.tile([128, 2 * TC], F32, tag=f"mv{tag}", name=f"mv_{tag}")
        for c in range(TC):
            st = p_st.tile([128, 6], F32, tag="st", name=f"st_{tag}")
            nc.vector.bn_stats(out=st, in_=x_ts[c])
            nc.vector.bn_aggr(out=mv[:, 2 * c : 2 * c + 2], in_=st)
        mv3 = mv.rearrange("p (c two) -> p c two", two=2)
        rstd = rsqrt_dve(mv3[:, :, 1], TC, tag)  # [128, TC]
        nmr = p_st.tile([128, TC], F32, tag=f"nm{tag}", name=f"nm_{tag}")
        nc.vector.scalar_tensor_tensor(
            out=nmr, in0=mv3[:, :, 0], scalar=-1.0, in1=rstd, op0=OP.mult, op1=OP.mult
        )
        h_ts = []
        for c in range(TC):
            h_t = pool.tile([128, D], BF16, tag=tag, name=f"h_{tag}")
            nc.vector.tensor_scalar(
                out=h_t,
                in0=x_ts[c],
                scalar1=rstd[:, c : c + 1],
                scalar2=nmr[:, c : c + 1],
                op0=OP.mult,
                op1=OP.add,
            )
            if gb_nontriv:
                nc.vector.tensor_mul(out=h_t, in0=h_t, in1=g_b)
                nc.vector.tensor_add(out=h_t, in0=h_t, in1=be_b)
            h_ts.append(h_t)
        return h_ts

    def transpose_3(h_ts, tag, copy_engines):
        """token-major [128, D] x TC -> feature-major f32r [128, NT] x KC.

        All TC transposes of one k-chunk land in a single PSUM tile so each
        chunk needs one PSUM->SBUF copy; copies rotate over copy_engines.
        """
        hT = [
            p_hT.tile([128, NT], F32R, tag=f"{tag}{k}", name=f"hT_{tag}{k}")
            for k in range(KC)
        ]
        for k in range(KC):
            pt_ = ps.tile([128, NT], BF16, tag="ps", name=f"tp_{tag}")
            for c in range(TC):
                nc.tensor.transpose(
                    out=pt_[:, c * 128 : (c + 1) * 128],
                    in_=h_ts[c][:, k * 128 : (k + 1) * 128],
                    identity=ident,
                )
            eng = copy_engines[k % len(copy_engines)]
            if eng == "act":
                nc.scalar.copy(out=hT[k], in_=pt_)
            elif eng == "dve":
                nc.vector.tensor_copy(out=hT[k], in_=pt_)
            else:
                nc.gpsimd.tensor_copy(out=hT[k], in_=pt_)
        return hT

    # x loads: one wide DMA per tile, prefetched 2 tiles ahead so the SP
    # sequencer never head-of-line blocks future loads behind output stores.
    x_tiles = {}

    def load_x(i):
        if i >= n_tiles:
            return
        t = p_x.tile([128, TC * D], BF16, tag="x", name="x")
        nc.sync.dma_start(
            out=t.rearrange("p (c d) -> p c d", c=TC),
            in_=x_dr[i * NT : (i + 1) * NT, :].rearrange("(c p) d -> p c d", p=128),
        )
        x_tiles[i] = t

    load_x(0)
    load_x(1)

    for it in range(n_tiles):
        row0 = it * NT
        load_x(it + 2)

        x_big = x_tiles.pop(it)
        x_ts = [x_big[:, c * D : (c + 1) * D] for c in range(TC)]

        h_ts = layernorm_group(x_ts, g1_b, be1_b, flags["g1be1"], p_h, "h")
        hT = transpose_3(h_ts, "hT", ("act", "act", "act"))

        # ---- Q,K feature-major, 64-padded heads: chunk ch = heads (2ch, 2ch+1) ----
        qk_sb = []  # [qi][ch] -> [128, NT]
        for qi in range(2):
            row = []
            for ch in range(4):
                pm = ps.tile([128, NT], F32, tag="ps", name="qk_ps")
                for k in range(KC):
                    nc.tensor.matmul(
                        out=pm,
                        lhsT=wqk[(qi, k, ch)],
                        rhs=hT[k],
                        start=(k == 0),
                        stop=(k == KC - 1),
                    )
                sb = p_qk.tile([128, NT], BF16, tag=f"qk{qi}{ch}", name=f"qk{qi}{ch}")
                if flags["bqk"]:
                    nc.scalar.activation(
                        out=sb,
                        in_=pm,
                        func=AF.Identity,
                        bias=bqk[:, qi * 4 + ch : qi * 4 + ch + 1],
                        scale=1.0,
                    )
                else:
                    # GPSIMD can't read PSUM; split PSUM->SBUF copies Act/DVE
                    nc.scalar.copy(out=sb, in_=pm)
                row.append(sb)
            qk_sb.append(row)

        # ---- V token-major [128 tok, DP] (64-padded heads) + half-swapped copy ----
        v_sb, vs_sb = [], []
        for c in range(TC):
            pm = ps.tile([128, DP], F32, tag="ps", name="v_ps")
            for k in range(KC):
                nc.tensor.matmul(
                    out=pm,
                    lhsT=hT[k][:, c * 128 : (c + 1) * 128],
                    rhs=wv[k],
                    start=(k == 0),
                    stop=(k == KC - 1),
                )
            sb = p_v.tile([128, DP], BF16, tag=f"v{c}", name=f"v{c}")
            nc.vector.tensor_copy(out=sb, in_=pm)
            if flags["bv"]:
                nc.vector.tensor_add(out=sb, in0=sb, in1=bv_b)
            # half-swapped copy on GpSimd (SBUF->SBUF is legal there)
            sw = p_v.tile([128, DP], BF16, tag=f"vs{c}", name=f"vs{c}")
            nc.gpsimd.tensor_copy(out=sw[0:64, :], in_=sb[64:128, :])
            nc.gpsimd.tensor_copy(out=sw[64:128, :], in_=sb[0:64, :])
            v_sb.append(sb)
            vs_sb.append(sw)

        # ---- attention, per batch-pair p; at_ps bank (g, cg) = [128, NT] ----
        at_ps = {
            (g, cg): ps_at.tile([128, NT], F32, tag=f"at{g}{cg}", name=f"at{g}{cg}")
            for g in range(2)
            for cg in range(2)
        }
        # Pass 1: all scores + softmax. PE emits every p's score matmuls
        # before any probs transpose, so softmax(p) (Act exp -> GpSimd
        # mask/renorm -> DVE reduce) overlaps scores(p+1..) instead of
        # stalling the PE stream.
        exs = []
        for p in range(NB // 2):
            # scores split into two PSUM banks by head parity: a PSUM bank
            # must only be written by ONE PE row-tile (= lhsT base) at a time.
            sc_par = [
                ps.tile([128, 4 * T], F32, tag="ps", name=f"sc_ps{par}")
                for par in range(2)
            ]
            for half in range(2):
                bb = 2 * p + half
                for h in range(H):
                    ch, off = h // 2, EP * (h % 2)
                    nc.tensor.matmul(
                        out=sc_par[h % 2][
                            64 * half : 64 * half + 64, (h // 2) * T : (h // 2 + 1) * T
                        ],
                        lhsT=qk_sb[0][ch][off : off + E, bb * T : (bb + 1) * T],
                        rhs=qk_sb[1][ch][off : off + E, bb * T : (bb + 1) * T],
                        start=True,
                        stop=True,
                    )
            # ex layout: col of head h = (h%2)*256 + (h//2)*64
            ex = p_sm.tile([128, 8 * T], BF16, tag="ex", name="ex")
            for par in range(2):
                nc.scalar.activation(
                    out=ex[:, par * 4 * T : (par + 1) * 4 * T],
                    in_=sc_par[par],
                    func=AF.Exp,
                    bias=0.0,
                    scale=INV_SQRT_E,
                )
            nc.gpsimd.tensor_mul(out=ex, in0=ex, in1=mask)
            rs = p_st.tile([128, H], F32, tag="rsum", name="rsum")
            nc.vector.reduce_sum(
                out=rs,
                in_=ex.rearrange("p (h s) -> p h s", h=H),
                axis=mybir.AxisListType.X,
            )
            rr = p_st.tile([128, H], F32, tag="rrec", name="rrec")
            nc.vector.reciprocal(out=rr, in_=rs)
            for h in range(H):
                nc.gpsimd.tensor_scalar_mul(
                    out=ex[:, h * T : (h + 1) * T],
                    in0=ex[:, h * T : (h + 1) * T],
                    scalar1=rr[:, h : h + 1],
                )
            exs.append(ex)

        # Pass 2: probs transposes + attnV per p.
        for p in range(NB // 2):
            ex = exs[p]
            # transpose probs: 4x [128,128] PE transposes -> one PSUM tile,
            # one PSUM->SBUF copy. Block j2 covers heads j in {2j2, 2j2+1}.
            ptp = ps.tile([128, 8 * T], BF16, tag="ps", name="pt_ps")
            for j2 in range(4):
                nc.tensor.transpose(
                    out=ptp[:, j2 * 128 : (j2 + 1) * 128],
                    in_=ex[:, j2 * 128 : (j2 + 1) * 128],
                    identity=ident,
                )
            ptsb = p_pt.tile([128, 8 * T], BF16, tag="pt", name="pt")
            nc.vector.tensor_copy(out=ptsb, in_=ptp)
            # attnV. probsT block for head h (ex col j=(h%2)*4 + h//2):
            #   partitions 64*(j%2) .. +64 (s), free (j//2)*128 + 64*half + t.
            # lhsT (V rows of bb) must sit at the same partition base 64*(j%2):
            # use v_sb when j%2 == bb%2 else the half-swapped copy.
            # at_ps bank (g=ch%2, cg=ch//2): per bank ONE lhsT row tile
            # (64*(j%2) with j%2 == ch%2 == g).
            for half in range(2):
                bb = 2 * p + half
                c, hb = bb // 2, 64 * (bb % 2)
                for h in range(H):
                    ch = h // 2
                    j = (h % 2) * 4 + ch
                    pbase = 64 * (j % 2)
                    vt = v_sb[c] if (j % 2) == (bb % 2) else vs_sb[c]
                    nc.tensor.matmul(
                        out=at_ps[(ch % 2, ch // 2)][
                            EP * (h % 2) : EP * (h % 2) + EP,
                            bb * T : (bb + 1) * T,
                        ],
                        lhsT=vt[pbase : pbase + 64, h * EP : (h + 1) * EP],
                        rhs=ptsb[
                            pbase : pbase + 64,
                            (j // 2) * 128 + hb : (j // 2) * 128 + hb + 64,
                        ],
                        start=True,
                        stop=True,
                    )
        at_sb = {}
        for gi, (g, cg) in enumerate(at_ps):
            sb = p_at.tile([128, NT], F32R, tag=f"atsb{g}{cg}", name=f"atsb{g}{cg}")
            if gi % 2 == 0:
                nc.scalar.copy(out=sb, in_=at_ps[(g, cg)])
            else:
                nc.vector.tensor_copy(out=sb, in_=at_ps[(g, cg)])
            at_sb[(g, cg)] = sb

        # ---- Wo (token-major out) + residual ----
        hr_ts = []
        for c in range(TC):
            pm = ps.tile([128, D], F32, tag="ps", name="wo_ps")
            for ch in range(4):
                nc.tensor.matmul(
                    out=pm,
                    lhsT=at_sb[(ch % 2, ch // 2)][:, c * 128 : (c + 1) * 128],
                    rhs=wo[ch],
                    start=(ch == 0),
                    stop=(ch == 3),
                )
            hr = p_hr.tile([128, D], BF16, tag="hr", name="hr")
            nc.vector.tensor_add(out=hr, in0=pm, in1=h_ts[c])
            if flags["bo"]:
                nc.vector.tensor_add(out=hr, in0=hr, in1=bo_b)
            hr_ts.append(hr)

        h2_ts = layernorm_group(hr_ts, g2_b, be2_b, flags["g2be2"], p_h2, "h2")
        h2T = transpose_3(h2_ts, "h2T", ("dve", "dve", "dve"))

        # ---- FFN1 + relu (split Act / DVE / GpSimd) ----
        rel = []
        for f in range(FC):
            pm = ps.tile([128, NT], F32, tag="ps", name="f1_ps")
            for k in range(KC):
                nc.tensor.matmul(
                    out=pm,
                    lhsT=w1[(k, f)],
                    rhs=h2T[k],
                    start=(k == 0),
                    stop=(k == KC - 1),
                )
            sb = p_rel.tile([128, NT], F32R, tag=f"rel{f}", name=f"rel{f}")
            if flags["b1"]:
                nc.scalar.activation(
                    out=sb, in_=pm, func=AF.Relu, bias=b1c[:, f : f + 1], scale=1.0
                )
            elif f % 2 == 0:
                nc.scalar.activation(out=sb, in_=pm, func=AF.Relu, bias=0.0, scale=1.0)
            else:
                nc.vector.tensor_scalar_max(out=sb, in0=pm, scalar1=0.0)
            rel.append(sb)

        # ---- FFN2 (token-major out) + residual + single wide store ----
        o_big = p_out.tile([128, TC * D], F32, tag="o", name="o")
        for c in range(TC):
            pm = ps.tile([128, D], F32, tag="ps", name="f2_ps")
            for f in range(FC):
                nc.tensor.matmul(
                    out=pm,
                    lhsT=rel[f][:, c * 128 : (c + 1) * 128],
                    rhs=w2[f],
                    start=(f == 0),
                    stop=(f == FC - 1),
                )
            o_t = o_big[:, c * D : (c + 1) * D]
            nc.vector.tensor_add(out=o_t, in0=pm, in1=h2_ts[c])
            if flags["b2"]:
                nc.vector.tensor_add(out=o_t, in0=o_t, in1=b2_b)
        nc.sync.dma_start(
            out=out_dr[row0 : row0 + NT, :].rearrange("(c p) d -> p c d", p=128),
            in_=o_big.rearrange("p (c d) -> p c d", c=TC),
        )

    ctx.close()


def prep_inputs(inputs, b_core):
    f32 = np.float32
    bf16 = mybir.dt.np(BF16)
    wq, wk, wvv = (np.asarray(inputs[k], f32) for k in ("wq", "wk", "wv"))
    bq, bk, bv = (np.asarray(inputs[k], f32) for k in ("bq", "bk", "bv"))
    wo, bo = np.asarray(inputs["wo"], f32), np.asarray(inputs["bo"], f32)
    w1, b1 = np.asarray(inputs["w1"], f32), np.asarray(inputs["b1"], f32)
    w2, b2 = np.asarray(inputs["w2"], f32), np.asarray(inputs["b2"], f32)
    g1, be1 = np.asarray(inputs["g1"], f32), np.asarray(inputs["be1"], f32)
    g2, be2 = np.asarray(inputs["g2"], f32), np.asarray(inputs["be2"], f32)

    # wqk[qi, k, ch] = [128, 128]: cols 0:48 head 2ch, 64:112 head 2ch+1, rest 0
    wqk = np.zeros((2, KC, 4, 128, 128), f32)
    for qi, w in enumerate((wq, wk)):
        for k in range(KC):
            for ch in range(4):
                wqk[qi, k, ch, :, 0:E] = w[2 * ch][k * 128 : (k + 1) * 128, :]
                wqk[qi, k, ch, :, EP : EP + E] = w[2 * ch + 1][k * 128 : (k + 1) * 128, :]
    bqk = np.zeros((128, 8), f32)
    for qi, b in enumerate((bq, bk)):
        for ch in range(4):
            bqk[0:E, qi * 4 + ch] = b[2 * ch]
            bqk[EP : EP + E, qi * 4 + ch] = b[2 * ch + 1]

    # wv padded: [KC, 128, DP] cols h*64+e
    wv_p = np.zeros((KC, 128, DP), f32)
    for k in range(KC):
        for h in range(H):
            wv_p[k, :, h * EP : h * EP + E] = wvv[h][k * 128 : (k + 1) * 128, :]
    bv_b = np.zeros((DP,), f32)
    for h in range(H):
        bv_b[h * EP : h * EP + E] = bv[h]

    # wo chunks: [4, 128, D]; rows = 64-padded head-pair (2ch, 2ch+1), pads zero
    wo_c = np.zeros((4, 128, D), f32)
    for ch in range(4):
        wo_c[ch, 0:E, :] = wo[(2 * ch) * E : (2 * ch + 1) * E, :]
        wo_c[ch, EP : EP + E, :] = wo[(2 * ch + 1) * E : (2 * ch + 2) * E, :]

    w1_c = np.zeros((KC, FC, 128, 128), f32)
    for k in range(KC):
        for f in range(FC):
            w1_c[k, f] = w1[k * 128 : (k + 1) * 128, f * 128 : (f + 1) * 128]
    b1c = np.zeros((128, FC), f32)
    for f in range(FC):
        b1c[:, f] = b1[f * 128 : (f + 1) * 128]
    w2_c = np.stack([w2[f * 128 : (f + 1) * 128, :] for f in range(FC)])

    mask = np.tile(np.tril(np.ones((T, T), f32)), (2, H))  # [128, 8*64]

    quake = np.zeros((128, 8), np.int32)
    quake[:, 0:4] = QUAKE_MAGIC
    quake[:, 4:8] = 1

    bcast = lambda v, w: np.broadcast_to(v[None, :], (128, w)).copy()

    flags = {
        "g1be1": bool(np.any(g1 != 1) or np.any(be1 != 0)),
        "g2be2": bool(np.any(g2 != 1) or np.any(be2 != 0)),
        "bqk": bool(np.any(bq) or np.any(bk)),
        "bv": bool(np.any(bv)),
        "bo": bool(np.any(bo)),
        "b1": bool(np.any(b1)),
        "b2": bool(np.any(b2)),
    }
    common = dict(
        ident=np.eye(128, dtype=f32).astype(bf16),
        mask=mask.astype(bf16),
        wqk=wqk,
        wv=wv_p,
        wo=wo_c,
        w1=w1_c,
        w2=w2_c,
        bqk=bqk,
        bv_b=bcast(bv_b, DP),
        b1c=b1c,
        g1_b=bcast(g1, D),
        be1_b=bcast(be1, D),
        g2_b=bcast(g2, D),
        be2_b=bcast(be2, D),
        bo_b=bcast(bo, D),
        b2_b=bcast(b2, D),
        quake=quake,
    )
    return common, flags


CONST_SHAPES = dict(
    ident=(128, 128),
    mask=(128, 8 * T),
    wqk=(2, KC, 4, 128, 128),
    wv=(KC, 128, DP),
    wo=(4, 128, D),
    w1=(KC, FC, 128, 128),
    w2=(FC, 128, D),
    bqk=(128, 8),
    bv_b=(128, DP),
    b1c=(128, FC),
    g1_b=(128, D),
    be1_b=(128, D),
    g2_b=(128, D),
    be2_b=(128, D),
    bo_b=(128, D),
    b2_b=(128, D),
    quake=(128, 8),
)


BF16_NAMES = {"mask", "ident", "x"}
F32R_NAMES = {"wqk", "wv", "wo", "w1", "w2"}


def build_program(b_core, flags):
    from concourse import bacc

    nc = bacc.Bacc("TRN2", target_bir_lowering=False, debug=False)
    aps = {}
    for name, sh in {**CONST_SHAPES, "x": (b_core, T, D)}.items():
        if name in BF16_NAMES:
            dt = BF16
        elif name in F32R_NAMES:
            dt = F32R
        elif name == "quake":
            dt = I32
        else:
            dt = F32
        aps[name] = nc.dram_tensor(name, list(sh), dt, kind="ExternalInput").ap()
    aps["out"] = nc.dram_tensor("out", [b_core, T, D], F32, kind="ExternalOutput").ap()
    aps["flags"] = flags
    with tile.TileContext(nc) as tc:
        build_body(tc, aps, b_core)
    nc.compile()
    return nc


LAST_EXEC_NS = None


def kernel(**inputs):
    global LAST_EXEC_NS
    from concourse.bass_utils import run_bass_kernel_spmd

    bf16 = mybir.dt.np(BF16)
    x = np.ascontiguousarray(np.asarray(inputs["x"], np.float32)).astype(bf16)
    common, flags = prep_inputs(inputs, B_CORE)
    nc = build_program(B_CORE, flags)
    in_maps = []
    for c in range(N_CORES):
        m = dict(common)
        m["x"] = np.ascontiguousarray(x[c * B_CORE : (c + 1) * B_CORE])
        in_maps.append(m)
    res = run_bass_kernel_spmd(nc, in_maps, core_ids=list(range(N_CORES)))
    LAST_EXEC_NS = res.exec_time_ns
    out = np.concatenate([r["out"] for r in res.results], axis=0)
    return out.astype(np.float32)


# revision 27
# speedup vs baseline: 1.8670x; 1.8670x over previous
"""Trainium2 Bass kernel for a dense transformer block (pre-LN, causal MHA + FFN).

Reference computation (per batch element b, T=64 tokens, D=384 features):
    h   = LN(x)*g1 + be1
    q,k,v per-head linears; scores = q k^T / sqrt(48); causal softmax
    attn = probs @ v, concat heads, @ wo + bo
    h    = h + attn              (residual from the *normed* x)
    h2   = LN(h)*g2 + be2
    out  = h2 + relu(h2@w1+b1)@w2 + b2

Sharding: pure data parallel over batch (2048 -> 256 per core, 8 cores),
params replicated; the same single-core program runs SPMD on all 8 cores.

v3 design (vs the fp32/f32r baseline):
  - ALL matmul operands bf16 (PSUM accumulation fp32): 1 cyc/row at any
    output width (fp32 = 4 cyc/row; f32r = 4 cyc/row below 256-wide).
  - NB=8 batch elems per tile (512 tokens): wider moving dims amortize the
    per-instruction issue cost (bf16 matmuls lower to Ldweights+Matmult
    pairs on the PE sequencer) and halve DMA/LN/copy instruction counts.
  - LN rstd via Quake rsqrt (bitcast + shift + 1 Newton step) entirely on
    DVE: the Act engine then only runs Exp/Copy/Relu which share one
    activation table (Sqrt shares no table with Exp -> the baseline
    reloaded act tables twice per tile, 1.28us each).
  - transposes on PE (bf16 transpose = 1 cyc/row), pairs merged into one
    PSUM tile so each [128,512] needs a single PSUM->SBUF copy.
  - V "half-swapped" copy on DVE (bf16 SBUF->SBUF runs at 2-4x).
  - PSUM->SBUF copies balanced across Act / DVE / GpSimd.

Per-core layout (tile = NB=8 batch elems = 512 tokens):
  - token-major [128 tok, feat] for LayerNorm (bn_stats) + residual adds
  - feature-major [feat 128-chunk, tok] for matmul inputs
  - heads 64-padded so per-(batch,head) matmuls sit at partition base 0/64
    (legal 64x64 PE array tiling; tile_position inferred from AP bases).
"""

import sys

sys.path.insert(0, "/opt/trn_rl_repo")

import numpy as np

import concourse.bass as bass
import concourse.tile as tile
from concourse import mybir

# ---- problem constants (hardcoded per contract) ----
B_TOTAL = 2048
T = 64
D = 384
H = 8
E = 48  # head size
EP = 64  # padded head size
F = 4 * D  # ffn hidden 1536
N_CORES = 8
B_CORE = B_TOTAL // N_CORES  # 256
LN_EPS = 1e-5
INV_SQRT_E = float(E) ** -0.5

NB = 8  # batch elems per tile
NT = NB * T  # tokens per tile = 512
KC = D // 128  # 3 contraction chunks for D
FC = F // 128  # 12 chunks for ffn hidden
TC = NT // 128  # 4 token chunks per tile
DP = H * EP  # padded qkv width 512

F32 = mybir.dt.float32
F32R = mybir.dt.float32r
I32 = mybir.dt.int32
BF16 = mybir.dt.bfloat16

QUAKE_MAGIC = 0x5F3759DF


def build_body(tc, aps, b_core):
    from contextlib import ExitStack

    ctx = ExitStack()
    nc = tc.nc
    n_tiles = b_core * T // NT

    x_dr = aps["x"].rearrange("b t d -> (b t) d")
    out_dr = aps["out"].rearrange("b t d -> (b t) d")

    AF = mybir.ActivationFunctionType
    OP = mybir.AluOpType
    flags = aps["flags"]

    singles = ctx.enter_context(tc.tile_pool(name="singles", bufs=1))

    def load_const(name, shape, src_ap, dt=BF16):
        t_ = singles.tile(list(shape), dt, name=f"sb_{name}")
        nc.sync.dma_start(out=t_, in_=src_ap)
        return t_

    ident = load_const("ident", [128, 128], aps["ident"])
    mask = load_const("mask", [128, 8 * T], aps["mask"])
    # dense-path weights in f32r: self-loading matmuls (no Ldweights on the
    # PE sequencer) and 1 cyc/row since every dense output is >= 384 wide.
    wqk = {
        (qi, k, ch): load_const(
            f"wqk{qi}{k}{ch}", [128, 128], aps["wqk"][qi, k, ch], F32R
        )
        for qi in range(2)
        for k in range(KC)
        for ch in range(4)
    }
    wv = {k: load_const(f"wv{k}", [128, DP], aps["wv"][k], F32R) for k in range(KC)}
    wo = {ch: load_const(f"wo{ch}", [128, D], aps["wo"][ch], F32R) for ch in range(4)}
    w1 = {
        (k, f): load_const(f"w1{k}_{f}", [128, 128], aps["w1"][k, f], F32R)
        for k in range(KC)
        for f in range(FC)
    }
    w2 = {f: load_const(f"w2{f}", [128, D], aps["w2"][f], F32R) for f in range(FC)}
    # bias/affine constants only when the problem actually uses them
    bqk = load_const("bqk", [128, 8], aps["bqk"], F32) if flags["bqk"] else None
    bv_b = load_const("bv_b", [128, DP], aps["bv_b"], F32) if flags["bv"] else None
    b1c = load_const("b1c", [128, FC], aps["b1c"], F32) if flags["b1"] else None
    g1_b = load_const("g1_b", [128, D], aps["g1_b"], F32) if flags["g1be1"] else None
    be1_b = load_const("be1_b", [128, D], aps["be1_b"], F32) if flags["g1be1"] else None
    g2_b = load_const("g2_b", [128, D], aps["g2_b"], F32) if flags["g2be2"] else None
    be2_b = load_const("be2_b", [128, D], aps["be2_b"], F32) if flags["g2be2"] else None
    bo_b = load_const("bo_b", [128, D], aps["bo_b"], F32) if flags["bo"] else None
    b2_b = load_const("b2_b", [128, D], aps["b2_b"], F32) if flags["b2"] else None
    quake = load_const("quake", [128, 8], aps["quake"], I32)  # cols 0:4=magic, 4:8=1

    pool = lambda nm, n, **kw: ctx.enter_context(tc.tile_pool(name=nm, bufs=n, **kw))
    ps = pool("ps", 4, space="PSUM")  # 1 tag -> 4 banks
    ps_at = pool("ps_at", 1, space="PSUM")  # 4 tags -> 4 banks
    p_x = pool("p_x", 4)
    p_h = pool("p_h", 6)
    p_hT = pool("p_hT", 1)  # 3 tags; writer phase directly precedes readers
    p_qk = pool("p_qk", 2)  # 8 tags
    p_v = pool("p_v", 2)  # 8 tags
    p_sm = pool("p_sm", 5)  # 4 ex tiles alive across pass1/pass2 + pipeline
    p_pt = pool("p_pt", 3)
    p_at = pool("p_at", 1)  # 4 tags
    p_hr = pool("p_hr", 6)
    p_h2 = pool("p_h2", 6)
    p_rel = pool("p_rel", 1)  # 12 tags
    p_out = pool("p_out", 3)
    p_st = pool("p_st", 8)

    def rsqrt_dve(ve_view, n, tag):
        """rstd[128, n] = 1/sqrt(ve) via Quake bitcast + 1 Newton step on DVE.

        Keeps Sqrt off the Act engine so Exp/Copy/Relu share one act table.
        ve in [~0.5, ~2] here, so one Newton step reaches ~2e-3 rel err.
        """
        ve = p_st.tile([128, n], F32, tag=f"q0{tag}", name=f"q0{tag}")
        nc.vector.tensor_scalar_add(out=ve, in0=ve_view, scalar1=LN_EPS)
        t1 = p_st.tile([128, n], I32, tag=f"q1{tag}", name=f"q1{tag}")
        nc.vector.tensor_tensor(
            out=t1, in0=ve.bitcast(I32), in1=quake[:, 4 : 4 + n], op=OP.logical_shift_right
        )
        y0 = p_st.tile([128, n], F32, tag=f"q2{tag}", name=f"q2{tag}")
        nc.vector.tensor_tensor(
            out=y0.bitcast(I32), in0=quake[:, 0:n], in1=t1, op=OP.subtract
        )
        y = y0
        for ni in range(2):  # two Newton steps: rel err ~3e-6
            a = p_st.tile([128, n], F32, tag=f"q3{tag}{ni}", name=f"q3{tag}{ni}")
            nc.vector.tensor_mul(out=a, in0=y, in1=y)
            b = p_st.tile([128, n], F32, tag=f"q4{tag}{ni}", name=f"q4{tag}{ni}")
            nc.vector.scalar_tensor_tensor(
                out=b, in0=a, scalar=-0.5, in1=ve, op0=OP.mult, op1=OP.mult
            )
            c = p_st.tile([128, n], F32, tag=f"q5{tag}{ni}", name=f"q5{tag}{ni}")
            nc.vector.tensor_scalar_add(out=c, in0=b, scalar1=1.5)
            yn = p_st.tile([128, n], F32, tag=f"q6{tag}{ni}", name=f"q6{tag}{ni}")
            nc.vector.tensor_mul(out=yn, in0=y, in1=c)
            y = yn
        return y

    def layernorm_group(x_ts, g_b, be_b, gb_nontriv, pool, tag):
        """LN over TC token chunks; per-chunk stats, one batched rstd."""
        mv = p_st.tile([128, 2 * TC], F32, tag=f"mv{tag}", name=f"mv_{tag}")
        for c in range(TC):
            st = p_st.tile([128, 6], F32, tag="st", name=f"st_{tag}")
            nc.vector.bn_stats(out=st, in_=x_ts[c])
            nc.vector.bn_aggr(out=mv[:, 2 * c : 2 * c + 2], in_=st)
        mv3 = mv.rearrange("p (c two) -> p c two", two=2)
        rstd = rsqrt_dve(mv3[:, :, 1], TC, tag)  # [128, TC]
        nmr = p_st.tile([128, TC], F32, tag=f"nm{tag}", name=f"nm_{tag}")
        nc.vector.scalar_tensor_tensor(
            out=nmr, in0=mv3[:, :, 0], scalar=-1.0, in1=rstd, op0=OP.mult, op1=OP.mult
        )
        h_ts = []
        for c in range(TC):
            h_t = pool.tile([128, D], BF16, tag=tag, name=f"h_{tag}")
            nc.vector.tensor_scalar(
                out=h_t,
                in0=x_ts[c],
                scalar1=rstd[:, c : c + 1],
                scalar2=nmr[:, c : c + 1],
                op0=OP.mult,
                op1=OP.add,
            )
            if gb_nontriv:
                nc.vector.tensor_mul(out=h_t, in0=h_t, in1=g_b)
                nc.vector.tensor_add(out=h_t, in0=h_t, in1=be_b)
            h_ts.append(h_t)
        return h_ts

    def transpose_3(h_ts, tag, copy_engines):
        """token-major [128, D] x TC -> feature-major f32r [128, NT] x KC.

        All TC transposes of one k-chunk land in a single PSUM tile so each
        chunk needs one PSUM->SBUF copy; copies rotate over copy_engines.
        """
        hT = [
            p_hT.tile([128, NT], F32R, tag=f"{tag}{k}", name=f"hT_{tag}{k}")
            for k in range(KC)
        ]
        for k in range(KC):
            pt_ = ps.tile([128, NT], BF16, tag="ps", name=f"tp_{tag}")
            for c in range(TC):
                nc.tensor.transpose(
                    out=pt_[:, c * 128 : (c + 1) * 128],
                    in_=h_ts[c][:, k * 128 : (k + 1) * 128],
                    identity=ident,
                )
            eng = copy_engines[k % len(copy_engines)]
            if eng == "act":
                nc.scalar.copy(out=hT[k], in_=pt_)
            else:
                nc.vector.tensor_copy(out=hT[k], in_=pt_)
        return hT

    # x loads: one wide DMA per tile, prefetched 2 tiles ahead so the SP
    # sequencer never head-of-line blocks future loads behind output stores.
    x_tiles = {}

    def load_x(i):
        if i >= n_tiles:
            return
        t = p_x.tile([128, TC * D], BF16, tag="x", name="x")
        nc.sync.dma_start(
            out=t.rearrange("p (c d) -> p c d", c=TC),
            in_=x_dr[i * NT : (i + 1) * NT, :].rearrange("(c p) d -> p c d", p=128),
        )
        x_tiles[i] = t

    load_x(0)
    load_x(1)

    for it in range(n_tiles):
        row0 = it * NT
        load_x(it + 2)

        x_big = x_tiles.pop(it)
        x_ts = [x_big[:, c * D : (c + 1) * D] for c in range(TC)]

        h_ts = layernorm_group(x_ts, g1_b, be1_b, flags["g1be1"], p_h, "h")
        hT = transpose_3(h_ts, "hT", ("act", "act", "act"))

        # ---- Q,K feature-major, 64-padded heads: chunk ch = heads (2ch, 2ch+1) ----
        qk_sb = []  # [qi][ch] -> [128, NT]
        for qi in range(2):
            row = []
            for ch in range(4):
                pm = ps.tile([128, NT], F32, tag="ps", name="qk_ps")
                for k in range(KC):
                    nc.tensor.matmul(
                        out=pm,
                        lhsT=wqk[(qi, k, ch)],
                        rhs=hT[k],
                        start=(k == 0),
                        stop=(k == KC - 1),
                    )
                sb = p_qk.tile([128, NT], BF16, tag=f"qk{qi}{ch}", name=f"qk{qi}{ch}")
                if flags["bqk"]:
                    nc.scalar.activation(
                        out=sb,
                        in_=pm,
                        func=AF.Identity,
                        bias=bqk[:, qi * 4 + ch : qi * 4 + ch + 1],
                        scale=1.0,
                    )
                else:
                    # GPSIMD can't read PSUM; split PSUM->SBUF copies Act/DVE
                    nc.scalar.copy(out=sb, in_=pm)
                row.append(sb)
            qk_sb.append(row)

        # ---- V token-major [128 tok, DP] (64-padded heads) + half-swapped copy ----
        v_sb, vs_sb = [], []
        for c in range(TC):
            pm = ps.tile([128, DP], F32, tag="ps", name="v_ps")
            for k in range(KC):
                nc.tensor.matmul(
                    out=pm,
                    lhsT=hT[k][:, c * 128 : (c + 1) * 128],
                    rhs=wv[k],
                    start=(k == 0),
                    stop=(k == KC - 1),
                )
            sb = p_v.tile([128, DP], BF16, tag=f"v{c}", name=f"v{c}")
            nc.vector.tensor_copy(out=sb, in_=pm)
            if flags["bv"]:
                nc.vector.tensor_add(out=sb, in0=sb, in1=bv_b)
            # half-swapped copy (bf16 SBUF->SBUF on DVE runs at 2-4x)
            sw = p_v.tile([128, DP], BF16, tag=f"vs{c}", name=f"vs{c}")
            nc.vector.tensor_copy(out=sw[0:64, :], in_=sb[64:128, :])
            nc.vector.tensor_copy(out=sw[64:128, :], in_=sb[0:64, :])
            v_sb.append(sb)
            vs_sb.append(sw)

        # ---- attention, per batch-pair p; at_ps bank (g, cg) = [128, NT] ----
        at_ps = {
            (g, cg): ps_at.tile([128, NT], F32, tag=f"at{g}{cg}", name=f"at{g}{cg}")
            for g in range(2)
            for cg in range(2)
        }
        # Pass 1: all scores + softmax. PE emits every p's score matmuls
        # before any probs transpose, so softmax(p) (Act exp -> GpSimd
        # mask/renorm -> DVE reduce) overlaps scores(p+1..) instead of
        # stalling the PE stream.
        exs = []
        for p in range(NB // 2):
            # scores split into two PSUM banks by head parity: a PSUM bank
            # must only be written by ONE PE row-tile (= lhsT base) at a time.
            sc_par = [
                ps.tile([128, 4 * T], F32, tag="ps", name=f"sc_ps{par}")
                for par in range(2)
            ]
            for half in range(2):
                bb = 2 * p + half
                for h in range(H):
                    ch, off = h // 2, EP * (h % 2)
                    nc.tensor.matmul(
                        out=sc_par[h % 2][
                            64 * half : 64 * half + 64, (h // 2) * T : (h // 2 + 1) * T
                        ],
                        lhsT=qk_sb[0][ch][off : off + E, bb * T : (bb + 1) * T],
                        rhs=qk_sb[1][ch][off : off + E, bb * T : (bb + 1) * T],
                        start=True,
                        stop=True,
                    )
            # ex layout: col of head h = (h%2)*256 + (h//2)*64
            ex = p_sm.tile([128, 8 * T], BF16, tag="ex", name="ex")
            for par in range(2):
                nc.scalar.activation(
                    out=ex[:, par * 4 * T : (par + 1) * 4 * T],
                    in_=sc_par[par],
                    func=AF.Exp,
                    bias=0.0,
                    scale=INV_SQRT_E,
                )
            nc.vector.tensor_mul(out=ex, in0=ex, in1=mask)
            rs = p_st.tile([128, H], F32, tag="rsum", name="rsum")
            nc.vector.reduce_sum(
                out=rs,
                in_=ex.rearrange("p (h s) -> p h s", h=H),
                axis=mybir.AxisListType.X,
            )
            rr = p_st.tile([128, H], F32, tag="rrec", name="rrec")
            nc.vector.reciprocal(out=rr, in_=rs)
            for h in range(H):
                nc.vector.tensor_scalar_mul(
                    out=ex[:, h * T : (h + 1) * T],
                    in0=ex[:, h * T : (h + 1) * T],
                    scalar1=rr[:, h : h + 1],
                )
            exs.append(ex)

        # Pass 2: probs transposes + attnV per p.
        for p in range(NB // 2):
            ex = exs[p]
            # transpose probs: 4x [128,128] PE transposes -> one PSUM tile,
            # one PSUM->SBUF copy. Block j2 covers heads j in {2j2, 2j2+1}.
            ptp = ps.tile([128, 8 * T], BF16, tag="ps", name="pt_ps")
            for j2 in range(4):
                nc.tensor.transpose(
                    out=ptp[:, j2 * 128 : (j2 + 1) * 128],
                    in_=ex[:, j2 * 128 : (j2 + 1) * 128],
                    identity=ident,
                )
            ptsb = p_pt.tile([128, 8 * T], BF16, tag="pt", name="pt")
            nc.vector.tensor_copy(out=ptsb, in_=ptp)
            # attnV. probsT block for head h (ex col j=(h%2)*4 + h//2):
            #   partitions 64*(j%2) .. +64 (s), free (j//2)*128 + 64*half + t.
            # lhsT (V rows of bb) must sit at the same partition base 64*(j%2):
            # use v_sb when j%2 == bb%2 else the half-swapped copy.
            # at_ps bank (g=ch%2, cg=ch//2): per bank ONE lhsT row tile
            # (64*(j%2) with j%2 == ch%2 == g).
            for half in range(2):
                bb = 2 * p + half
                c, hb = bb // 2, 64 * (bb % 2)
                for h in range(H):
                    ch = h // 2
                    j = (h % 2) * 4 + ch
                    pbase = 64 * (j % 2)
                    vt = v_sb[c] if (j % 2) == (bb % 2) else vs_sb[c]
                    nc.tensor.matmul(
                        out=at_ps[(ch % 2, ch // 2)][
                            EP * (h % 2) : EP * (h % 2) + EP,
                            bb * T : (bb + 1) * T,
                        ],
                        lhsT=vt[pbase : pbase + 64, h * EP : (h + 1) * EP],
                        rhs=ptsb[
                            pbase : pbase + 64,
                            (j // 2) * 128 + hb : (j // 2) * 128 + hb + 64,
                        ],
                        start=True,
                        stop=True,
                    )
        at_sb = {}
        for gi, (g, cg) in enumerate(at_ps):
            sb = p_at.tile([128, NT], F32R, tag=f"atsb{g}{cg}", name=f"atsb{g}{cg}")
            if gi % 2 == 0:
                nc.scalar.copy(out=sb, in_=at_ps[(g, cg)])
            else:
                nc.vector.tensor_copy(out=sb, in_=at_ps[(g, cg)])
            at_sb[(g, cg)] = sb

        # ---- Wo (token-major out) + residual ----
        hr_ts = []
        for c in range(TC):
            pm = ps.tile([128, D], F32, tag="ps", name="wo_ps")
            for ch in range(4):
                nc.tensor.matmul(
                    out=pm,
                    lhsT=at_sb[(ch % 2, ch // 2)][:, c * 128 : (c + 1) * 128],
                    rhs=wo[ch],
                    start=(ch == 0),
                    stop=(ch == 3),
                )
            hr = p_hr.tile([128, D], BF16, tag="hr", name="hr")
            nc.vector.tensor_add(out=hr, in0=pm, in1=h_ts[c])
            if flags["bo"]:
                nc.vector.tensor_add(out=hr, in0=hr, in1=bo_b)
            hr_ts.append(hr)

        h2_ts = layernorm_group(hr_ts, g2_b, be2_b, flags["g2be2"], p_h2, "h2")
        h2T = transpose_3(h2_ts, "h2T", ("dve", "dve", "dve"))

        # ---- FFN1 + relu (split Act / DVE / GpSimd) ----
        rel = []
        for f in range(FC):
            pm = ps.tile([128, NT], F32, tag="ps", name="f1_ps")
            for k in range(KC):
                nc.tensor.matmul(
                    out=pm,
                    lhsT=w1[(k, f)],
                    rhs=h2T[k],
                    start=(k == 0),
                    stop=(k == KC - 1),
                )
            sb = p_rel.tile([128, NT], F32R, tag=f"rel{f}", name=f"rel{f}")
            if flags["b1"]:
                nc.scalar.activation(
                    out=sb, in_=pm, func=AF.Relu, bias=b1c[:, f : f + 1], scale=1.0
                )
            elif f % 2 == 0:
                nc.scalar.activation(out=sb, in_=pm, func=AF.Relu, bias=0.0, scale=1.0)
            else:
                nc.vector.tensor_scalar_max(out=sb, in0=pm, scalar1=0.0)
            rel.append(sb)

        # ---- FFN2 (token-major out) + residual + single wide store ----
        o_big = p_out.tile([128, TC * D], F32, tag="o", name="o")
        for c in range(TC):
            pm = ps.tile([128, D], F32, tag="ps", name="f2_ps")
            for f in range(FC):
                nc.tensor.matmul(
                    out=pm,
                    lhsT=rel[f][:, c * 128 : (c + 1) * 128],
                    rhs=w2[f],
                    start=(f == 0),
                    stop=(f == FC - 1),
                )
            o_t = o_big[:, c * D : (c + 1) * D]
            nc.vector.tensor_add(out=o_t, in0=pm, in1=h2_ts[c])
            if flags["b2"]:
                nc.vector.tensor_add(out=o_t, in0=o_t, in1=b2_b)
        nc.sync.dma_start(
            out=out_dr[row0 : row0 + NT, :].rearrange("(c p) d -> p c d", p=128),
            in_=o_big.rearrange("p (c d) -> p c d", c=TC),
        )

    ctx.close()


def prep_inputs(inputs, b_core):
    f32 = np.float32
    bf16 = mybir.dt.np(BF16)
    wq, wk, wvv = (np.asarray(inputs[k], f32) for k in ("wq", "wk", "wv"))
    bq, bk, bv = (np.asarray(inputs[k], f32) for k in ("bq", "bk", "bv"))
    wo, bo = np.asarray(inputs["wo"], f32), np.asarray(inputs["bo"], f32)
    w1, b1 = np.asarray(inputs["w1"], f32), np.asarray(inputs["b1"], f32)
    w2, b2 = np.asarray(inputs["w2"], f32), np.asarray(inputs["b2"], f32)
    g1, be1 = np.asarray(inputs["g1"], f32), np.asarray(inputs["be1"], f32)
    g2, be2 = np.asarray(inputs["g2"], f32), np.asarray(inputs["be2"], f32)

    # wqk[qi, k, ch] = [128, 128]: cols 0:48 head 2ch, 64:112 head 2ch+1, rest 0
    wqk = np.zeros((2, KC, 4, 128, 128), f32)
    for qi, w in enumerate((wq, wk)):
        for k in range(KC):
            for ch in range(4):
                wqk[qi, k, ch, :, 0:E] = w[2 * ch][k * 128 : (k + 1) * 128, :]
                wqk[qi, k, ch, :, EP : EP + E] = w[2 * ch + 1][k * 128 : (k + 1) * 128, :]
    bqk = np.zeros((128, 8), f32)
    for qi, b in enumerate((bq, bk)):
        for ch in range(4):
            bqk[0:E, qi * 4 + ch] = b[2 * ch]
            bqk[EP : EP + E, qi * 4 + ch] = b[2 * ch + 1]

    # wv padded: [KC, 128, DP] cols h*64+e
    wv_p = np.zeros((KC, 128, DP), f32)
    for k in range(KC):
        for h in range(H):
            wv_p[k, :, h * EP : h * EP + E] = wvv[h][k * 128 : (k + 1) * 128, :]
    bv_b = np.zeros((DP,), f32)
    for h in range(H):
        bv_b[h * EP : h * EP + E] = bv[h]

    # wo chunks: [4, 128, D]; rows = 64-padded head-pair (2ch, 2ch+1), pads zero
    wo_c = np.zeros((4, 128, D), f32)
    for ch in range(4):
        wo_c[ch, 0:E, :] = wo[(2 * ch) * E : (2 * ch + 1) * E, :]
        wo_c[ch, EP : EP + E, :] = wo[(2 * ch + 1) * E : (2 * ch + 2) * E, :]

    w1_c = np.zeros((KC, FC, 128, 128), f32)
    for k in range(KC):
        for f in range(FC):
            w1_c[k, f] = w1[k * 128 : (k + 1) * 128, f * 128 : (f + 1) * 128]
    b1c = np.zeros((128, FC), f32)
    for f in range(FC):
        b1c[:, f] = b1[f * 128 : (f + 1) * 128]
    w2_c = np.stack([w2[f * 128 : (f + 1) * 128, :] for f in range(FC)])

    mask = np.tile(np.tril(np.ones((T, T), f32)), (2, H))  # [128, 8*64]

    quake = np.zeros((128, 8), np.int32)
    quake[:, 0:4] = QUAKE_MAGIC
    quake[:, 4:8] = 1

    bcast = lambda v, w: np.broadcast_to(v[None, :], (128, w)).copy()

    flags = {
        "g1be1": bool(np.any(g1 != 1) or np.any(be1 != 0)),
        "g2be2": bool(np.any(g2 != 1) or np.any(be2 != 0)),
        "bqk": bool(np.any(bq) or np.any(bk)),
        "bv": bool(np.any(bv)),
        "bo": bool(np.any(bo)),
        "b1": bool(np.any(b1)),
        "b2": bool(np.any(b2)),
    }
    common = dict(
        ident=np.eye(128, dtype=f32).astype(bf16),
        mask=mask.astype(bf16),
        wqk=wqk,
        wv=wv_p,
        wo=wo_c,
        w1=w1_c,
        w2=w2_c,
        bqk=bqk,
        bv_b=bcast(bv_b, DP),
        b1c=b1c,
        g1_b=bcast(g1, D),
        be1_b=bcast(be1, D),
        g2_b=bcast(g2, D),
        be2_b=bcast(be2, D),
        bo_b=bcast(bo, D),
        b2_b=bcast(b2, D),
        quake=quake,
    )
    return common, flags


CONST_SHAPES = dict(
    ident=(128, 128),
    mask=(128, 8 * T),
    wqk=(2, KC, 4, 128, 128),
    wv=(KC, 128, DP),
    wo=(4, 128, D),
    w1=(KC, FC, 128, 128),
    w2=(FC, 128, D),
    bqk=(128, 8),
    bv_b=(128, DP),
    b1c=(128, FC),
    g1_b=(128, D),
    be1_b=(128, D),
    g2_b=(128, D),
    be2_b=(128, D),
    bo_b=(128, D),
    b2_b=(128, D),
    quake=(128, 8),
)


BF16_NAMES = {"mask", "ident", "x"}
F32R_NAMES = {"wqk", "wv", "wo", "w1", "w2"}


def build_program(b_core, flags):
    from concourse import bacc

    nc = bacc.Bacc("TRN2", target_bir_lowering=False, debug=False)
    aps = {}
    for name, sh in {**CONST_SHAPES, "x": (b_core, T, D)}.items():
        if name in BF16_NAMES:
            dt = BF16
        elif name in F32R_NAMES:
            dt = F32R
        elif name == "quake":
            dt = I32
        else:
            dt = F32
        aps[name] = nc.dram_tensor(name, list(sh), dt, kind="ExternalInput").ap()
    aps["out"] = nc.dram_tensor("out", [b_core, T, D], F32, kind="ExternalOutput").ap()
    aps["flags"] = flags
    with tile.TileContext(nc) as tc:
        build_body(tc, aps, b_core)
    nc.compile()
    return nc


LAST_EXEC_NS = None


def kernel(**inputs):
    global LAST_EXEC_NS
    from concourse.bass_utils import run_bass_kernel_spmd

    bf16 = mybir.dt.np(BF16)
    x = np.ascontiguousarray(np.asarray(inputs["x"], np.float32)).astype(bf16)
    common, flags = prep_inputs(inputs, B_CORE)
    nc = build_program(B_CORE, flags)
    in_maps = []
    for c in range(N_CORES):
        m = dict(common)
        m["x"] = np.ascontiguousarray(x[c * B_CORE : (c + 1) * B_CORE])
        in_maps.append(m)
    res = run_bass_kernel_spmd(nc, in_maps, core_ids=list(range(N_CORES)))
    LAST_EXEC_NS = res.exec_time_ns
    out = np.concatenate([r["out"] for r in res.results], axis=0)
    return out.astype(np.float32)
